# revision 29
# baseline (speedup 1.0000x reference)
"""Trainium2 Bass kernel for a 2-layer GCN encoder (GCNConv -> ReLU -> {GCNConv mu, GCNConv logstd}).

Strategy (8 NeuronCores, SPMD):
  - Math: propagate(M) = D^-1/2 (A+I) D^-1/2 M == d * ((A+I) @ (d * M)) with d = deg^-1/2,
    so per-edge norm weights disappear: scale rows by d before and after message passing.
  - Layers 2 and 3 share the propagate: fuse W_mu/W_logstd into one [128,128] matmul + one
    message-passing pass over 128 features, split on the host afterwards.
  - Sharding: nodes are partitioned across the 8 cores (dst-sharding). Each core owns
    NPC = N/8 output rows and processes the ~E/8 edges pointing into them.
  - Node tables are laid out in shard-padded rows: node v lives at table row
    (v // NPC) * SH + (v % NPC) with SH = T*128, so each core's phase-A/phase-C shard is
    128-aligned AND equals its dst range. Self-loop edges are NOT gathered: each finalize
    adds the locally-kept own-row tile instead.
  - Layer-1 linear is sharded (each core computes its own rows), one AllGather publishes
    the table; same for the layer-2 table. AllGather outputs are plain (Local) DRAM so
    dma_gather can source them directly (no Shared-tensor bounce copies).
  - Message passing: async single-packet dma_gather (int16 idx, 1024 rows per call, 4 SWDGE
    queues) pulls source rows from the HBM table; a one-hot matrix (DVE is_equal vs iota)
    turns the segment-sum into PE matmuls accumulated in PSUM per 128-dst-node tile.
  - int16 gather indices only address <=32767 rows, so tables are split in two halves
    (AP offsets into one tensor) and each core's edge list is bucketed by source half
    (then by dst tile, padded to multiples of 128 with dummy edges whose one-hot column
    is out of range).
  - All cores run the same program (SPMD): per-(tile,half) group sizes are padded to the
    max over cores.

kernel(**inputs) takes the full-size inputs and returns (mu, logstd) as float32 numpy arrays.
"""
import sys

sys.path.insert(0, "/opt/trn_rl_repo")

import numpy as np
import ml_dtypes

import concourse.bass as bass
import concourse.bacc as bacc
import concourse.mybir as mybir
import concourse.tile as tile
from concourse.bass_utils import run_bass_kernel_spmd

BF16 = ml_dtypes.bfloat16

# ---------------- configuration ----------------
FULL_CFG = dict(
    n=50000,        # nodes
    fin=512,        # input features
    hid=128,        # hidden features
    out2=128,       # fused mu+logstd features
    n_cores=8,
    blk=512,        # phase-A block width
    g_edges=1024,   # gather super-chunk (edges per dma_gather; >1024 breaks single_packet)
    single_packet=True,  # fire-and-forget, DMA drains async
    swdge_queues=4,
    gather_bufs=10,
    oh_bufs=4,
    dma_scratch=32768,  # SWDGE ring carveout: 2 in-flight gathers per queue
    table_bf16=True,
    mm1_bf16=True,
    reps=1,         # kernel body repetitions (for timing)
)


def _ceil(a, b):
    return -(-a // b)


def _dims(cfg):
    N, C = cfg["n"], cfg["n_cores"]
    NPC = N // C
    T = _ceil(NPC, 128)
    SH = T * 128          # shard rows per core (128-aligned, >= NPC)
    NPAD = C * SH
    # tables are published as two sub-tables (= the int16 gather halves):
    # A holds every core's first SA shard rows, B the remaining SB rows.
    # Each sub-table gets its own AllGather so half-A gathers can start
    # while half B is still collecting.
    SA = ((T + 1) // 2) * 128
    SB = SH - SA
    HS = C * SA
    assert HS <= 32767 and C * SB <= 32767
    return NPC, T, SH, NPAD, HS, SA, SB


def preprocess(cfg, x, edge_index, W1, b1, W_mu, b_mu, W_logstd, b_logstd):
    """Host-side: degrees, edge bucketing/padding, operand staging. Returns
    (meta, in_maps). Pure index/layout work plus parameter reformatting."""
    N, C = cfg["n"], cfg["n_cores"]
    NPC, T, SH, NPAD, HS, SA, SB = _dims(cfg)
    G = cfg["g_edges"]
    t_dt = BF16 if cfg["table_bf16"] else np.float32
    m_dt = BF16 if cfg["mm1_bf16"] else np.float32

    x = np.asarray(x, np.float32)
    ei = np.asarray(edge_index).astype(np.int64)
    W1 = np.asarray(W1, np.float32)
    b1 = np.asarray(b1, np.float32)
    Wcat = np.concatenate([np.asarray(W_mu, np.float32), np.asarray(W_logstd, np.float32)], axis=0)
    bcat = np.concatenate([np.asarray(b_mu, np.float32), np.asarray(b_logstd, np.float32)], axis=0)

    # self-loops are handled locally in the finalizers, not gathered
    src, dst = ei[0], ei[1]
    deg = (np.bincount(dst, minlength=N) + 1.0).astype(np.float32)  # in-degree + self
    dvec = (1.0 / np.sqrt(deg)).astype(np.float32)

    sq = src % NPC
    scr = src // NPC
    half = (sq >= SA).astype(np.int64)
    # half-local table row of each edge's source
    rows_loc = np.where(half == 0, scr * SA + sq, scr * SB + (sq - SA))
    core = dst // NPC
    tloc = (dst % NPC) // 128
    key = (core * T + tloc) * 2 + half
    order = np.argsort(key, kind="stable")
    ks, rs, ds = key[order], rows_loc[order], dst[order]
    counts = np.bincount(ks, minlength=C * T * 2).reshape(C, T, 2)
    gpad = ((counts.max(axis=0) + 127) // 128) * 128  # [T, 2] padded group sizes
    Lh = gpad.sum(axis=0)  # per-half padded edge totals (same for all cores)
    cpt = (gpad // 128)    # chunks per (tile, half)
    offs = np.concatenate([[0], np.cumsum(counts.reshape(-1))])

    w1t = np.ascontiguousarray(W1.T).astype(m_dt)          # [fin, hid]
    wcatt = np.ascontiguousarray(Wcat.T).astype(t_dt)      # [hid, out2]
    iota_arr = np.tile(np.arange(128), (128, 1)).astype(t_dt)
    ident = np.eye(128, dtype=t_dt)

    K_tot = int(Lh.sum() // 128)
    in_maps = []
    for c in range(C):
        xt_c = np.zeros((cfg["fin"], SH), m_dt)
        xt_c[:, :NPC] = x[c * NPC:(c + 1) * NPC].T
        # edge order: (tile, half) groups; gather idx streams are per half in
        # tile order; dstloc columns are (tile: half0-chunks then half1-chunks).
        bufs_ = [np.zeros(int(Lh[0]), np.int16), np.zeros(int(Lh[1]), np.int16)]
        ph = [0, 0]
        dstloc_all = np.full(int(Lh.sum()), 200, np.int32)
        pos = 0
        for t in range(T):
            for h in (0, 1):
                g = int(counts[c, t, h])
                o = int(offs[(c * T + t) * 2 + h])
                sl = slice(o, o + g)
                bufs_[h][ph[h]:ph[h] + g] = rs[sl].astype(np.int16)
                dstloc_all[pos:pos + g] = (ds[sl] % NPC) - t * 128
                ph[h] += int(gpad[t, h])
                pos += int(gpad[t, h])
        idx_h = [np.tile(b.reshape(-1, 16).T, (8, 1)).copy() if b.size
                 else np.zeros((128, 1), np.int16) for b in bufs_]
        dstloc_arr = np.ascontiguousarray(dstloc_all.reshape(-1, 128).T).astype(t_dt)

        d_own_pad = np.ones(SH, np.float32)
        d_own_pad[:NPC] = dvec[c * NPC:(c + 1) * NPC]
        d_rep = np.tile(d_own_pad, (128, 1)).astype(np.float32)  # [128, SH]

        in_maps.append({
            "xt": xt_c, "w1t": w1t, "wcatt": wcatt,
            "b1c": b1.reshape(-1, 1).copy(), "bcatc": bcat.reshape(-1, 1).copy(),
            "drep": d_rep,
            "iota": iota_arr, "ident": ident,
            "idx0": idx_h[0], "idx1": idx_h[1], "dstloc": dstloc_arr,
        })

    meta = dict(cpt=cpt.tolist(), Lh=[int(Lh[0]), int(Lh[1])], K_tot=K_tot)
    return meta, in_maps


def build_program(cfg, meta):
    N, C = cfg["n"], cfg["n_cores"]
    NPC, T, SH, NPAD, HS, SA, SB = _dims(cfg)
    FIN, HID, O2 = cfg["fin"], cfg["hid"], cfg["out2"]
    BLK, G = cfg["blk"], cfg["g_edges"]
    KC = FIN // 128
    cpt, Lh = meta["cpt"], meta["Lh"]
    K_tot = meta["K_tot"]
    dt_tab = mybir.dt.bfloat16 if cfg["table_bf16"] else mybir.dt.float32
    dt_mm = mybir.dt.bfloat16 if cfg["mm1_bf16"] else mybir.dt.float32
    f32 = mybir.dt.float32

    nc = bacc.Bacc("TRN2", target_bir_lowering=False, debug=False, num_devices=C,
                   num_swdge_queues=cfg.get("swdge_queues", 1),
                   dynamic_dma_scratch_size=cfg.get("dma_scratch", 16384))

    xt_d = nc.dram_tensor("xt", [FIN, SH], dt_mm, kind="ExternalInput")
    w1t_d = nc.dram_tensor("w1t", [FIN, HID], dt_mm, kind="ExternalInput")
    wcatt_d = nc.dram_tensor("wcatt", [HID, O2], dt_tab, kind="ExternalInput")
    b1c_d = nc.dram_tensor("b1c", [HID, 1], f32, kind="ExternalInput")
    bcatc_d = nc.dram_tensor("bcatc", [O2, 1], f32, kind="ExternalInput")
    drep_d = nc.dram_tensor("drep", [128, SH], f32, kind="ExternalInput")
    iota_d = nc.dram_tensor("iota", [128, 128], dt_tab, kind="ExternalInput")
    ident_d = nc.dram_tensor("ident", [128, 128], dt_tab, kind="ExternalInput")
    idx_d = [nc.dram_tensor(f"idx{h}", [128, max(Lh[h] // 16, 1)], mybir.dt.int16,
                            kind="ExternalInput") for h in (0, 1)]
    dstloc_d = nc.dram_tensor("dstloc", [128, max(K_tot, 1)], dt_tab, kind="ExternalInput")

    g1s_d = nc.dram_tensor("g1s", [SH, HID], dt_tab)
    # Local-output AllGather: the result is directly dma_gather-able (gathers
    # cannot source a Shared-address tensor), so no bounce copies are needed.
    # Two sub-tables per layer, one collective each (overlap + int16 halves).
    g1f_d = [nc.dram_tensor("g1fa", [C * SA, HID], dt_tab),
             nc.dram_tensor("g1fb", [C * SB, HID], dt_tab)]
    g2s_d = nc.dram_tensor("g2s", [SH, HID], dt_tab)
    g2f_d = [nc.dram_tensor("g2fa", [C * SA, HID], dt_tab),
             nc.dram_tensor("g2fb", [C * SB, HID], dt_tab)]
    outt_d = nc.dram_tensor("outt", [O2, SH], dt_tab, kind="ExternalOutput")

    with tile.TileContext(nc, trace_sim=bool(cfg.get("trace_sim"))) as tc:
        with tc.tile_pool(name="const", bufs=1) as const_p:
            w1t_sb = []
            for kc in range(KC):
                w = const_p.tile([128, HID], dt_mm, tag=f"w1t{kc}")
                nc.sync.dma_start(w[:], w1t_d[kc * 128:(kc + 1) * 128, :])
                w1t_sb.append(w)
            wcatt_sb = const_p.tile([HID, O2], dt_tab, tag="wcatt")
            nc.sync.dma_start(wcatt_sb[:], wcatt_d[:])
            b1_sb = const_p.tile([HID, 1], f32, tag="b1")
            nc.sync.dma_start(b1_sb[:], b1c_d[:])
            bcat_sb = const_p.tile([O2, 1], f32, tag="bcat")
            nc.sync.dma_start(bcat_sb[:], bcatc_d[:])
            drep_sb = const_p.tile([128, SH], f32, tag="drep")
            nc.sync.dma_start(drep_sb[:], drep_d[:])
            iota_sb = const_p.tile([128, 128], dt_tab, tag="iota")
            nc.sync.dma_start(iota_sb[:], iota_d[:])
            ident_sb = const_p.tile([128, 128], dt_tab, tag="ident")
            nc.sync.dma_start(ident_sb[:], ident_d[:])
            idx_sb = []
            for h in (0, 1):
                t_ = const_p.tile([128, max(Lh[h] // 16, 1)], mybir.dt.int16, tag=f"idx{h}")
                nc.sync.dma_start(t_[:], idx_d[h][:])
                idx_sb.append(t_)
            dstloc_sb = const_p.tile([128, max(K_tot, 1)], dt_tab, tag="dstloc")
            nc.sync.dma_start(dstloc_sb[:], dstloc_d[:])

            for _rep in range(cfg.get("reps", 1)):
                with nc.named_scope("body"):
                    _emit_body(nc, tc, cfg, meta, locals())

    nc.compile()
    return nc


def _emit_body(nc, tc, cfg, meta, env):
    """One full forward pass. `env` carries the SBUF constants + DRAM handles."""
    N, C = cfg["n"], cfg["n_cores"]
    NPC, T, SH, NPAD, HS, SA, SB = _dims(cfg)
    FIN, HID, O2 = cfg["fin"], cfg["hid"], cfg["out2"]
    BLK, G = cfg["blk"], cfg["g_edges"]
    KC = FIN // 128
    SPC = G // 128
    cpt, Lh = meta["cpt"], meta["Lh"]
    dt_tab = mybir.dt.bfloat16 if cfg["table_bf16"] else mybir.dt.float32
    dt_mm = mybir.dt.bfloat16 if cfg["mm1_bf16"] else mybir.dt.float32
    f32 = mybir.dt.float32
    AF = mybir.ActivationFunctionType
    OP = mybir.AluOpType

    xt_d, dstloc_sb, idx_sb = env["xt_d"], env["dstloc_sb"], env["idx_sb"]
    g2s_d, g2f_d, outt_d = env["g2s_d"], env["g2f_d"], env["outt_d"]
    g1s_d, g1f_d = env["g1s_d"], env["g1f_d"]
    w1t_sb, wcatt_sb = env["w1t_sb"], env["wcatt_sb"]
    b1_sb, bcat_sb = env["b1_sb"], env["bcat_sb"]
    drep_sb = env["drep_sb"]
    iota_sb, ident_sb = env["iota_sb"], env["ident_sb"]

    blks = ([BLK] * (SA // BLK) + ([SA % BLK] if SA % BLK else [])
            + [BLK] * (SB // BLK) + ([SB % BLK] if SB % BLK else []))
    NB = len(blks)

    with tc.tile_pool(name="own1", bufs=1) as own1_p, \
         tc.tile_pool(name="own2", bufs=1) as own2_p, \
         tc.tile_pool(name="htp", bufs=1) as ht_p, \
         tc.tile_pool(name="fin", bufs=4) as fin_p, \
         tc.tile_pool(name="pc_t", bufs=3) as ct_p:

        # t1ts[b]: own-row g1 tiles (feat-major, d-scaled) kept for fin1's
        # self-loop term; c_sc[b]: same for the layer-2 table, used in fin2.
        t1ts = [own1_p.tile([128, BLK], dt_tab, tag=f"t1ts{b}", name=f"t1ts{b}")
                for b in range(NB)]
        c_sc = [own2_p.tile([O2, BLK], dt_tab, tag=f"csc{b}", name=f"csc{b}")
                for b in range(NB)]
        ht_blk = [ht_p.tile([128, BLK], dt_tab, tag=f"htb{b}", name=f"htb{b}")
                  for b in range(NB)]

        # ---------------- phase A: g1 shard = d * (x @ W1.T) for own rows, AllGather
        sc = nc.named_scope("phaseA")
        sc.__enter__()
        with tc.tile_pool(name="pa_x", bufs=3) as xt_p, \
             tc.tile_pool(name="pa_w", bufs=3) as wst_p, \
             tc.tile_pool(name="pa_ps", bufs=2, space="PSUM") as pa, \
             tc.tile_pool(name="pa_ps2", bufs=2, space="PSUM") as pb:
            off = 0
            for b, bsz in enumerate(blks):
                xts = []
                for kc in range(KC):
                    xk = xt_p.tile([128, BLK], dt_mm, tag=f"xt{kc}")
                    nc.sync.dma_start(xk[:, :bsz], xt_d[kc * 128:(kc + 1) * 128,
                                                        off:off + bsz])
                    xts.append(xk)
                ps_a = pa.tile([128, BLK], f32, space="PSUM", tag="psa")
                for kc in range(KC):
                    nc.tensor.matmul(ps_a[:, :bsz], lhsT=w1t_sb[kc][:],
                                     rhs=xts[kc][:, :bsz],
                                     start=(kc == 0), stop=(kc == KC - 1))
                # d-scale applied pre-transpose; scaled own rows stay in SBUF
                nc.vector.tensor_tensor(out=t1ts[b][:, :bsz], in0=ps_a[:, :bsz],
                                        in1=drep_sb[:, off:off + bsz], op=OP.mult)
                sbn = bsz // 128
                wst = wst_p.tile([128, BLK // 128, HID], dt_tab, tag="wst")
                ps_b = pb.tile([128, BLK // 128, 128], dt_tab, space="PSUM", tag="psb")
                for s in range(sbn):
                    nc.tensor.transpose(ps_b[:, s, :],
                                        t1ts[b][:, s * 128:(s + 1) * 128], ident_sb[:])
                nc.scalar.copy(wst[:, :sbn, :], ps_b[:, :sbn, :])
                nc.sync.dma_start(
                    g1s_d[off:off + bsz, :].rearrange("(s p) f -> p s f", p=128),
                    wst[:, :sbn, :])
                off += bsz
                if off == SA:
                    nc.gpsimd.collective_compute(
                        "AllGather", mybir.AluOpType.bypass,
                        replica_groups=[list(range(C))],
                        ins=[g1s_d[0:SA, :]], outs=[g1f_d[0][:]])
            nc.gpsimd.collective_compute(
                "AllGather", mybir.AluOpType.bypass,
                replica_groups=[list(range(C))],
                ins=[g1s_d[SA:SH, :]], outs=[g1f_d[1][:]])
        sc.__exit__(None, None, None)
        if cfg.get("stop_after") == "A":
            return

        # ---------------- message passing (used for both layers)
        def propagate(tables, finalize, gathers_only=False):
            with tc.tile_pool(name="mp_g", bufs=cfg.get("gather_bufs", 2)) as gath_p, \
                 tc.tile_pool(name="mp_oh", bufs=cfg.get("oh_bufs", 4)) as oh_p, \
                 tc.tile_pool(name="mp_ps", bufs=4, space="PSUM") as psp:
                # issue gathers in the order their first chunk is consumed
                first_need = {}
                chk = [0, 0]
                pos = 0
                for t in range(T):
                    for h in (0, 1):
                        for _ in range(cpt[t][h]):
                            i = chk[h] // SPC
                            if (h, i) not in first_need:
                                first_need[(h, i)] = pos
                            chk[h] += 1
                            pos += 1
                nq = cfg.get("swdge_queues", 1)
                gh = {}
                qctr = 0
                for (h, i) in sorted(first_need, key=first_need.get):
                    n_i = min(G, Lh[h] - i * G)
                    gt = gath_p.tile([128, SPC, HID], dt_tab, tag=f"g{h}", name="gt")
                    nc.gpsimd.dma_gather(
                        out_ap=gt[:, :n_i // 128, :],
                        in_ap=tables[h],
                        idxs_ap=idx_sb[h][:, i * (G // 16):i * (G // 16) + n_i // 16],
                        num_idxs=n_i,
                        num_idxs_reg=n_i,
                        elem_size=HID,
                        single_packet=cfg.get("single_packet", True),
                        queue_num=qctr % nq,
                    )
                    qctr += 1
                    gh[(h, i)] = gt
                if gathers_only:
                    return
                # per tile: one one-hot build + one PSUM accumulation chain over
                # both halves' chunks (dstloc columns are laid out per tile as
                # [half-0 chunks, half-1 chunks] already).
                kh = [0, 0]
                kg = 0
                for t in range(T):
                    nch = cpt[t][0] + cpt[t][1]
                    if nch == 0:
                        z = oh_p.tile([128, 128], f32, tag="zero")
                        nc.vector.memset(z[:], 0.0)
                        finalize(t, z)
                        continue
                    oh = oh_p.tile([128, nch, 128], dt_tab, tag="oh", name="oh")
                    dsl = dstloc_sb[:, kg:kg + nch]
                    in0 = bass.AP(dsl.tensor, dsl.offset,
                                  [dsl.ap[0], [dsl.ap[1][0], nch], [0, 128]])
                    io = iota_sb[:]
                    in1 = bass.AP(io.tensor, io.offset, [io.ap[0], [0, nch], io.ap[1]])
                    nc.vector.tensor_tensor(out=oh[:], in0=in0, in1=in1, op=OP.is_equal)
                    ps_t = psp.tile([128, 128], f32, space="PSUM", tag="ps", name="ps_t")
                    j = 0
                    for h in (0, 1):
                        for _ in range(cpt[t][h]):
                            gt = gh[(h, kh[h] // SPC)]
                            nc.tensor.matmul(ps_t[:], lhsT=gt[:, kh[h] % SPC, :],
                                             rhs=oh[:, j, :],
                                             start=(j == 0), stop=(j == nch - 1))
                            kh[h] += 1
                            j += 1
                    finalize(t, ps_t)
                    kg += nch

        # tile t (own rows t*128..t*128+128) -> (block index, offset within block)
        tmap = {}
        _off = 0
        for _b, _bsz in enumerate(blks):
            for _s in range(_bsz // 128):
                tmap[(_off + _s * 128) // 128] = (_b, _s * 128)
            _off += _bsz

        # block boundaries: tile t is the LAST tile of its block -> emit the
        # phase-C work for that block right away (streams the layer-2 table
        # out during prop1; its collectives then overlap prop1's tail).
        blk_off = {}
        _off = 0
        for _b, _bsz in enumerate(blks):
            blk_off[_b] = _off
            _off += _bsz
        last_tile_of = {}
        for _t, (_b, _o) in tmap.items():
            if _o == blks[_b] - 128:
                last_tile_of[_b] = _t

        def emit_c_block(b, pc1, pc2):
            bsz = blks[b]
            off = blk_off[b]
            sbn = bsz // 128
            ps = pc1.tile([O2, BLK], f32, space="PSUM", tag="c1", name="c1")
            nc.tensor.matmul(ps[:, :bsz], lhsT=wcatt_sb[:],
                             rhs=ht_blk[b][:, :bsz], start=True, stop=True)
            # d-scale pre-transpose; scaled own rows stay in SBUF for fin2
            nc.vector.tensor_tensor(out=c_sc[b][:, :bsz], in0=ps[:, :bsz],
                                    in1=drep_sb[:, off:off + bsz], op=OP.mult)
            ps2 = pc2.tile([128, BLK // 128, O2], dt_tab, space="PSUM", tag="c2",
                           name="c2")
            for s in range(sbn):
                nc.tensor.transpose(ps2[:, s, :],
                                    c_sc[b][:, s * 128:(s + 1) * 128], ident_sb[:])
            g2t = ct_p.tile([128, BLK // 128, O2], dt_tab, tag="g2t", name="g2t")
            nc.scalar.copy(g2t[:, :sbn, :], ps2[:, :sbn, :])
            nc.sync.dma_start(
                g2s_d[off:off + bsz, :].rearrange("(s p) f -> p s f", p=128),
                g2t[:, :sbn, :])
            if off + bsz == SA:
                nc.gpsimd.collective_compute(
                    "AllGather", mybir.AluOpType.bypass,
                    replica_groups=[list(range(C))],
                    ins=[g2s_d[0:SA, :]], outs=[g2f_d[0][:]])
            if b == NB - 1:
                nc.gpsimd.collective_compute(
                    "AllGather", mybir.AluOpType.bypass,
                    replica_groups=[list(range(C))],
                    ins=[g2s_d[SA:SH, :]], outs=[g2f_d[1][:]])

        def make_fin1(pc1, pc2):
            def fin1(t, acc_t):
                b, o = tmap[t]
                tmp = fin_p.tile([128, 128], f32, tag="tmp")
                nc.vector.tensor_tensor(out=tmp[:], in0=acc_t[:],
                                        in1=t1ts[b][:, o:o + 128], op=OP.add)
                tmp2 = fin_p.tile([128, 128], f32, tag="tmp2")
                nc.vector.tensor_tensor(out=tmp2[:], in0=tmp[:],
                                        in1=drep_sb[:, t * 128:(t + 1) * 128],
                                        op=OP.mult)
                nc.scalar.activation(ht_blk[b][:, o:o + 128], tmp2[:],
                                     AF.Relu, bias=b1_sb[:])
                if last_tile_of.get(b) == t:
                    emit_c_block(b, pc1, pc2)
            return fin1

        with tc.tile_pool(name="pc_ps", bufs=2, space="PSUM") as pc1, \
             tc.tile_pool(name="pc_ps2", bufs=2, space="PSUM") as pc2:
            fin1 = make_fin1(pc1, pc2)
            if cfg.get("stop_after") == "G":
                propagate([g1f_d[0][:, :], g1f_d[1][:, :]], fin1, gathers_only=True)
                return
            with nc.named_scope("prop1"):
                propagate([g1f_d[0][:, :], g1f_d[1][:, :]], fin1)
        if cfg.get("stop_after") in ("P1", "C"):
            return

        # ---------------- phase D: second propagate + output
        def fin2(t, acc_t):
            b, o = tmap[t]
            tmp = fin_p.tile([128, 128], f32, tag="tmp")
            nc.vector.tensor_tensor(out=tmp[:], in0=acc_t[:],
                                    in1=c_sc[b][:, o:o + 128], op=OP.add)
            tmp2 = fin_p.tile([128, 128], f32, tag="tmp2")
            nc.vector.tensor_tensor(out=tmp2[:], in0=tmp[:],
                                    in1=drep_sb[:, t * 128:(t + 1) * 128], op=OP.mult)
            osb = fin_p.tile([O2, 128], dt_tab, tag="osb")
            nc.scalar.activation(osb[:], tmp2[:], AF.Identity, bias=bcat_sb[:])
            nc.sync.dma_start(outt_d[:, t * 128:(t + 1) * 128], osb[:])

        if cfg.get("stop_after") == "Dg1":
            propagate([g1f_d[0][:, :], g1f_d[1][:, :]], fin2)
        else:
            with nc.named_scope("prop2"):
                propagate([g2f_d[0][:, :], g2f_d[1][:, :]], fin2)


LAST_RESULTS = None


def run(cfg, x, edge_index, W1, b1, W_mu, b_mu, W_logstd, b_logstd, program_cache=None,
        trace=False, trace_cores=None):
    global LAST_RESULTS
    meta, in_maps = preprocess(cfg, x, edge_index, W1, b1, W_mu, b_mu, W_logstd, b_logstd)
    nc = build_program(cfg, meta)
    res = run_bass_kernel_spmd(nc, in_maps, list(range(cfg["n_cores"])),
                               trace=trace, trace_cores=trace_cores)
    LAST_RESULTS = res
    N, C = cfg["n"], cfg["n_cores"]
    NPC = N // C
    O = cfg["out2"] // 2
    mu = np.empty((N, O), np.float32)
    logstd = np.empty((N, O), np.float32)
    for c in range(C):
        ot = np.asarray(res.results[c]["outt"], np.float32)
        mu[c * NPC:(c + 1) * NPC] = ot[:O, :NPC].T
        logstd[c * NPC:(c + 1) * NPC] = ot[O:, :NPC].T
    return mu, logstd


def kernel(x, edge_index, W1, b1, W_mu, b_mu, W_logstd, b_logstd):
    mu, logstd = run(FULL_CFG, x, edge_index, W1, b1, W_mu, b_mu, W_logstd, b_logstd)
    return mu, logstd
